# revision 99
# baseline (speedup 1.0000x reference)
"""Trainium2 Bass kernel for nn_DiffusionModuleV2 (dense transformer block).

Sharding: 8 cores = 2 batches x 4 query-quarters; fully token-parallel
(AdaLN, projections, FFN on the core's own 384 tokens) with AllGathers
per 4-core batch group for K/V.

Device layout: transposed activations [D-partitions (6x128 chunks), token-free].
Attention is computed in the S^T layout: S^T[k, q] = K_kb^T @ Q per 128-key
block, so the softmaxed P^T feeds P@V directly as the moving operand (no
transposes).  The positional bias is applied multiplicatively
(P = exp(S) * exp(bias), exp(bias) gathered on host), and the softmax
denominator comes free from a ones-column appended to V.  Normalization is
batched at the attention tail (one reciprocal + per-head-pair PE broadcast).
g1/g2 gates and the AdaLN2 cond-side matmuls are precomputed during the
collective window to hide the AllGather latency.
"""

import sys

sys.path.insert(0, "/opt/trn_rl_repo")

import numpy as np
import ml_dtypes

BF = ml_dtypes.bfloat16
F8 = ml_dtypes.float8_e4m3
F32 = np.float32

B, N, D, H = 2, 1536, 768, 16
DH, DHP = 48, 64
FF = 4 * D
EPS = 1e-5
NCORES = 8
QPC = N // 4          # 384 queries per core
NCH = D // 128        # 6
FCH = FF // 128       # 24
HP = H // 2           # 8 head pairs
NKB = N // 128        # 12 key blocks of 128
QT = QPC // 128       # 3 token tiles of 128
VW = DH + 1           # 49: V columns + ones column per head

_PROGRAM_CACHE = {}


def ts(start, size):
    return slice(start, start + size)


# ----------------------------------------------------------------------------
# host-side layout helpers
# ----------------------------------------------------------------------------

def _chunkT(x_t):  # (D, T) -> [128, NCH, T]
    d, t = x_t.shape
    return np.ascontiguousarray(x_t.reshape(d // 128, 128, t).transpose(1, 0, 2))


def _wtiles(w):  # (Din, Cout) -> [128, Din/128, Cout/128, 128]
    din, cout = w.shape
    return np.ascontiguousarray(
        w.reshape(din // 128, 128, cout // 128, 128).transpose(1, 0, 2, 3)
    )


def _wtilesT(w):  # (Din, Cout) -> [128, Cout/128, Din/128, 128]  (co-major)
    din, cout = w.shape
    return np.ascontiguousarray(
        w.reshape(din // 128, 128, cout // 128, 128).transpose(1, 2, 0, 3)
    )


def _colvec(v):  # (D,) per-out-col bias -> [128, NCH, 1]
    return np.ascontiguousarray(v.reshape(NCH, 128, 1).transpose(1, 0, 2)).astype(F32)


def _rowvec(v):  # (D,) -> [1, NCH, 128]  (K=1 matmul lhsT slices)
    return np.ascontiguousarray(v.reshape(1, NCH, 128)).astype(F32)


def _pad_qk(w):  # (D, H*48) -> (D, H*64), head h cols at 64h..64h+47
    out = np.zeros((D, H * DHP), w.dtype)
    for h in range(H):
        out[:, h * DHP : h * DHP + DH] = w[:, h * DH : (h + 1) * DH]
    return out


def _pad_wo(w):  # (H*48, D) -> (H*64, D), head h rows at 64h..64h+47
    out = np.zeros((H * DHP, D), w.dtype)
    for h in range(H):
        out[h * DHP : h * DHP + DH, :] = w[h * DH : (h + 1) * DH, :]
    return out


def prep_weights(inputs):
    w = {}
    f = lambda k: np.asarray(inputs[k], np.float64)

    def adaln(pfx, ln_w, ln_b, gw, gb, bw):
        gw_eff = (ln_w[:, None] * gw).astype(BF)
        bw_eff = (ln_w[:, None] * bw).astype(BF)
        w[pfx + "gw"] = _wtiles(gw_eff)
        w[pfx + "bw"] = _wtiles(bw_eff)
        w[pfx + "gb"] = _colvec(gb + ln_b @ gw)
        w[pfx + "bb"] = _colvec(ln_b @ bw)
        w[pfx + "csg"] = _rowvec(-gw_eff.astype(np.float64).sum(0))
        w[pfx + "csb"] = _rowvec(-bw_eff.astype(np.float64).sum(0))

    adaln("a1", f("a1_ln_w"), f("a1_ln_b"), f("a1_gw"), f("a1_gb"), f("a1_bw"))
    adaln("a2", f("a2_ln_w"), f("a2_ln_b"), f("a2_gw"), f("a2_gb"), f("a2_bw"))

    # split the 1/sqrt(DH) between Q and K so both land in fp8's sweet spot
    w["wq"] = _wtilesT(_pad_qk((f("wq") * DH**-0.25).astype(BF)))
    w["wk"] = _wtiles(_pad_qk((f("wk") * DH**-0.25).astype(BF)))
    w["wv"] = _wtiles(f("wv").astype(BF))
    w["wg"] = _wtiles(f("wg").astype(BF))
    w["wo"] = _wtiles(_pad_wo(f("wo").astype(BF)))
    w["g1w"] = _wtiles(f("g1_w").astype(BF))
    w["g1b"] = _colvec(f("g1_b"))
    w["g2w"] = _wtiles(f("g2_w").astype(BF))
    w["g2b"] = _colvec(f("g2_b"))
    # SwiGLU weights in fp8 (DoubleRow), scaled x128 into e4m3's range;
    # the 1/128 is folded into the activation/gating ops on device.
    # co-major layout so the per-co streamed DMA slices are contiguous.
    w["swg"] = _wtilesT((f("sw_gate") * 128.0).astype(F8))
    w["swu"] = _wtilesT((f("sw_up") * 128.0).astype(F8))
    w["swd"] = _wtilesT((f("sw_down") * 128.0).astype(F8))

    # den-broadcast selectors: Dall row (hp) -> out partitions 0..47,
    # row (8+hp) -> out partitions 64..111
    selm = np.zeros((16, HP, 128), BF)
    for hp in range(HP):
        selm[hp, hp, 0:DH] = 1.0
        selm[8 + hp, hp, DHP : DHP + DH] = 1.0
    w["selm"] = selm
    return w


def host_prep(inputs):
    """Build the 8 per-core input maps (numpy, dtypes matching DRAM decls)."""
    wts = prep_weights(inputs)
    s = np.asarray(inputs["s"], F32)
    cond = np.asarray(inputs["s_cond"], F32)
    pw = np.asarray(inputs["pos_weight"], np.float64)  # (H, NBINS)
    expw = np.exp(pw).astype(F32)
    bins = np.asarray(inputs["pos_bins"])

    in_maps = []
    for c in range(NCORES):
        b, qi = c // 4, c % 4
        qsl = slice(qi * QPC, (qi + 1) * QPC)
        m = dict(wts)
        m["sT"] = _chunkT(s[b].T[:, qsl]).astype(BF)
        m["cT"] = _chunkT(cond[b].T[:, qsl]).astype(BF)
        m["sqT"] = _chunkT(s[b].T[:, qsl]).astype(F32)
        # E[h, k, kb, q] = exp(pw[h, bins[b, q, kb*128+k]])  (key-transposed)
        binsT = bins[b, qsl].T                    # (N keys, QPC queries)
        arr = expw[:, binsT]                      # (H, N, QPC)
        arr = arr.reshape(H, NKB, 128, QPC).transpose(0, 2, 1, 3)
        m["E"] = np.ascontiguousarray(arr.astype(BF))  # (H, 128, NKB, QPC)
        in_maps.append(m)
    return in_maps


def assemble_output(results):
    out = np.empty((B, N, D), F32)
    for c in range(NCORES):
        b, qi = c // 4, c % 4
        t = np.asarray(results[c]["outT"])  # [128, NCH, QPC]
        out[b, qi * QPC : (qi + 1) * QPC, :] = (
            t.transpose(1, 0, 2).reshape(D, QPC).T)
    return out


# ----------------------------------------------------------------------------
# device program
# ----------------------------------------------------------------------------

def declare_io(nc, mybir):
    f32, bf16 = mybir.dt.float32, mybir.dt.bfloat16
    dram = {}

    def din(name, shape, dt):
        dram[name] = nc.dram_tensor(name, shape, dt, kind="ExternalInput")

    din("sT", [128, NCH, QPC], bf16)
    din("cT", [128, NCH, QPC], bf16)
    din("sqT", [128, NCH, QPC], f32)
    din("E", [H, 128, NKB, QPC], bf16)
    din("selm", [16, HP, 128], bf16)
    for pfx in ("a1", "a2"):
        din(pfx + "gw", [128, NCH, NCH, 128], bf16)
        din(pfx + "bw", [128, NCH, NCH, 128], bf16)
        din(pfx + "gb", [128, NCH, 1], f32)
        din(pfx + "bb", [128, NCH, 1], f32)
        din(pfx + "csg", [1, NCH, 128], f32)
        din(pfx + "csb", [1, NCH, 128], f32)
    din("wq", [128, HP, NCH, 128], bf16)
    din("wk", [128, NCH, HP, 128], bf16)
    din("wv", [128, NCH, NCH, 128], bf16)
    din("wg", [128, NCH, NCH, 128], bf16)
    din("wo", [128, HP, NCH, 128], bf16)
    din("g1w", [128, NCH, NCH, 128], bf16)
    din("g1b", [128, NCH, 1], f32)
    din("g2w", [128, NCH, NCH, 128], bf16)
    din("g2b", [128, NCH, 1], f32)
    din("swg", [128, FCH, NCH, 128], mybir.dt.float8e4)
    din("swu", [128, FCH, NCH, 128], mybir.dt.float8e4)
    din("swd", [128, NCH, FCH, 128], mybir.dt.float8e4)
    dram["outT"] = nc.dram_tensor("outT", [128, NCH, QPC], f32,
                                  kind="ExternalOutput")
    return dram


def build_program():
    import concourse.mybir as mybir
    import concourse.tile as tile
    from concourse import bacc

    nc = bacc.Bacc("TRN2", target_bir_lowering=False, debug=False,
                   num_devices=NCORES)
    dram = declare_io(nc, mybir)
    with tile.TileContext(nc) as tc:
        _emit(nc, tc, dram, mybir)
    nc.compile()
    return nc


def _emit(nc, tc, dram, mybir):
    import contextlib

    from concourse.bass_isa import ReduceOp

    f32, bf16 = mybir.dt.float32, mybir.dt.bfloat16
    f8 = mybir.dt.float8e4
    AF = mybir.ActivationFunctionType
    OP = mybir.AluOpType

    ctx = contextlib.ExitStack()
    with ctx:
        const = ctx.enter_context(tc.tile_pool(name="const", bufs=1))
        outer = ctx.enter_context(tc.tile_pool(name="outer", bufs=1))

        # ---- constants / small residents ----
        onesmat = const.tile([128, 128], bf16, tag="onesmat")
        nc.vector.memset(onesmat[:], 1.0)
        cvec = {}
        for name in ("a1gb", "a1bb", "a2gb", "a2bb", "g1b", "g2b"):
            t = const.tile(list(dram[name].shape), dram[name].dtype,
                           name="c_" + name, tag=name)
            nc.sync.dma_start(out=t[:], in_=dram[name][:])
            cvec[name] = t

        selm_sb = const.tile([16, HP, 128], bf16, tag="selm")
        nc.sync.dma_start(out=selm_sb[:], in_=dram["selm"][:])
        eps128 = const.tile([128, 1], f32, tag="eps128")
        nc.vector.memset(eps128[:], EPS)

        # ---- persistent activations ----
        cT = outer.tile([128, NCH, QPC], bf16, tag="cT")
        for ci in range(NCH):
            nc.sync.dma_start(out=cT[:, ci, :], in_=dram["cT"][:, ci, :])
        s_new = outer.tile([128, NCH, QPC], f32, tag="s_new")
        Rs_c = outer.tile([128, QPC], f32, tag="Rs_c")

        # ------------------------------------------------------------------
        def ln_stats(x_bf, Mb, Rb, tag, sq_pre=None):
            """LN stats over the partition (D) axis via all-ones matmuls:
            ones.T @ x sums the partitions AND broadcasts the result to all
            128 rows in one full-activity PE instruction per chunk."""
            with tc.tile_pool(name="st_" + tag, bufs=1) as wp, \
                 tc.tile_pool(name="stp_" + tag, bufs=1, space="PSUM") as pp:
                psx = pp.tile([128, QPC], f32, tag="psx")
                pss = pp.tile([128, QPC], f32, tag="pss")
                for ci in range(NCH):
                    nc.tensor.matmul(psx[:], onesmat[:], x_bf[:, ci, :],
                                     start=(ci == 0), stop=(ci == NCH - 1))
                for ci in range(NCH):
                    if sq_pre is not None:
                        sq = sq_pre[:, ci, :]
                    else:
                        sqt = wp.tile([128, QPC], bf16, tag="sq", bufs=3)
                        nc.vector.tensor_mul(sqt[:], x_bf[:, ci, :],
                                             x_bf[:, ci, :])
                        sq = sqt[:]
                    nc.tensor.matmul(pss[:], onesmat[:], sq,
                                     start=(ci == 0), stop=(ci == NCH - 1))
                nc.vector.tensor_scalar_mul(Mb[:], psx[:], 1.0 / D)
                msq = wp.tile([128, QPC], f32, tag="msq")
                nc.vector.tensor_mul(msq[:], Mb[:], Mb[:])
                v = wp.tile([128, QPC], f32, tag="v")
                nc.vector.scalar_tensor_tensor(
                    v[:], pss[:], 1.0 / D, msq[:],
                    op0=OP.mult, op1=OP.subtract)
                lnv = wp.tile([128, QPC], f32, tag="lnv")
                nc.scalar.activation(lnv[:], v[:], AF.Ln, bias=eps128[:])
                nc.scalar.activation(Rb[:], lnv[:], AF.Exp, scale=-0.5)

        def ln_apply(x_bf, Mb, R_sb, xn, wp):
            """xn = (x - Mb) * R, with Mb/R already broadcast [128, T]."""
            for ch in range(NCH):
                d = wp.tile([128, QPC], f32, tag="d")
                nc.vector.tensor_sub(d[:], x_bf[:, ch, :], Mb[:])
                nc.vector.tensor_mul(xn[:, ch, :], d[:], R_sb[:])

        def adaln_gb(pfx, cn_t, xn, sn_out, gw_all, bw_all):
            """sn = sigmoid(psG + gb) * xn + (psB + bb), where
            psG/psB = W^T @ cn and cn = LN(cond) (scale folded into cn)."""
            gb, bb = cvec[pfx + "gb"], cvec[pfx + "bb"]
            with tc.tile_pool(name=pfx + "t", bufs=3) as tp, \
                 tc.tile_pool(name=pfx + "p", bufs=2, space="PSUM") as pp:
                for co in range(NCH):
                    gwc, bwc = gw_all[:, :, co, :], bw_all[:, :, co, :]
                    psg = pp.tile([128, QPC], f32, tag="psg")
                    psb = pp.tile([128, QPC], f32, tag="psb")
                    for ci in range(NCH):
                        nc.tensor.matmul(psg[:], gwc[:, ci, :],
                                         cn_t[:, ci, :],
                                         start=(ci == 0), stop=(ci == NCH - 1))
                        nc.tensor.matmul(psb[:], bwc[:, ci, :],
                                         cn_t[:, ci, :],
                                         start=(ci == 0), stop=(ci == NCH - 1))
                    sig = tp.tile([128, QPC], bf16, tag="sig")
                    nc.scalar.activation(sig[:], psg[:], AF.Sigmoid,
                                         bias=gb[:, co, :])
                    t1 = tp.tile([128, QPC], bf16, tag="t1")
                    nc.vector.tensor_mul(t1[:], sig[:], xn[:, co, :])
                    nc.vector.scalar_tensor_tensor(
                        sn_out[:, co, :], psb[:], bb[:, co, :],
                        t1[:], op0=OP.add, op1=OP.add)

        # ==================================================================
        # Phase A: AdaLN1 -> snT
        # ==================================================================
        attstack = contextlib.ExitStack()
        pAtt = attstack.enter_context(tc.tile_pool(name="pAtt", bufs=1))
        dp = attstack.enter_context(
            tc.tile_pool(name="ccd", bufs=1, space="DRAM"))
        cn = pAtt.tile([128, NCH, QPC], bf16, tag="cn")
        # K/Q zero-padded to full 128-row contraction: the wasted MACs are
        # free (matmul time is N-streaming bound) and the full-activity
        # matmuls keep the PE clock un-throttled.
        Kt128 = pAtt.tile([128, H, N], f8, tag="Kt128")
        Qt128 = pAtt.tile([128, H, QPC], f8, tag="Qt128")
        snstack = contextlib.ExitStack()
        pSn = snstack.enter_context(tc.tile_pool(name="pSn", bufs=1))
        snT = pSn.tile([128, NCH, QPC], bf16, tag="snT")
        # zero the attention pad rows early, off the DVE (GpSimd is idle here)
        nc.gpsimd.memset(Kt128[:], 0.0)
        nc.gpsimd.memset(Qt128[:], 0.0)
        with tc.tile_pool(name="pA", bufs=1) as pA:
            sT = pA.tile([128, NCH, QPC], bf16, tag="sT")
            for ci in range(NCH):
                nc.sync.dma_start(out=sT[:, ci, :], in_=dram["sT"][:, ci, :])
            a1gw_all = pA.tile([128, NCH, NCH, 128], bf16, tag="a1gw_all")
            nc.sync.dma_start(out=a1gw_all[:], in_=dram["a1gw"][:])
            a1bw_all = pA.tile([128, NCH, NCH, 128], bf16, tag="a1bw_all")
            nc.sync.dma_start(out=a1bw_all[:], in_=dram["a1bw"][:])
            xn = pA.tile([128, NCH, QPC], bf16, tag="xn")
            Rs_s = pA.tile([128, QPC], f32, tag="Rs_s")
            Mb_c = pA.tile([128, QPC], f32, tag="Mb_c")
            Mb_s = pA.tile([128, QPC], f32, tag="Mb_s")
            ln_stats(cT, Mb_c, Rs_c, "c")
            ln_stats(sT, Mb_s, Rs_s, "s")
            # normalized cond (LN sans affine; affine folded into weights),
            # reused by AdaLN1 + the AdaLN2 precompute
            with tc.tile_pool(name="bcAw", bufs=3) as bw:
                ln_apply(cT, Mb_c, Rs_c, cn, bw)
                ln_apply(sT, Mb_s, Rs_s, xn, bw)
            adaln_gb("a1", cn, xn, snT,
                     gw_all=a1gw_all, bw_all=a1bw_all)

        # ==================================================================
        # Phase B: projections + K/V AllGather + gate precompute
        # ==================================================================
        V49g = pAtt.tile([128, NKB, H, VW], bf16, tag="V49g")
        sig_g = pAtt.tile([128, NCH, QPC], bf16, tag="sig_g")
        sig1 = pAtt.tile([128, NCH, QPC], bf16, tag="sig1")
        gate12 = pAtt.tile([128, NCH, QPC], bf16, tag="gate12")
        sig2 = outer.tile([128, NCH, QPC], bf16, tag="sig2")
        psG2sb = outer.tile([128, NCH, QPC], bf16, tag="psG2sb")
        psB2sb = outer.tile([128, NCH, QPC], bf16, tag="psB2sb")

        with tc.tile_pool(name="pB", bufs=2) as pB, \
             tc.tile_pool(name="pBw", bufs=2) as pBw, \
             tc.tile_pool(name="pBp", bufs=2, space="PSUM") as pBp:
            KB = HP * QPC              # 3072
            VB = QT * H * VW           # 2352
            kc_in = dp.tile([96, KB], f8, name="kc_in")
            kc_out = dp.tile([4, 96, KB], f8, name="kc_out")
            vc_in = dp.tile([128, VB], f8, name="vc_in")
            vc_out = dp.tile([4, 128, VB], f8, name="vc_out")
            wk_all = pB.tile([128, NCH, HP, 128], bf16, tag="wk_all", bufs=1)
            nc.sync.dma_start(out=wk_all[:], in_=dram["wk"][:])
            wv_all = pB.tile([128, NCH, NCH, 128], bf16, tag="wv_all", bufs=1)
            nc.sync.dma_start(out=wv_all[:], in_=dram["wv"][:])
            Vf8 = pBw.tile([128, NKB, H, VW], f8, tag="w6")

            # ---- K projection, kick K AllGather ASAP (fp8, 96-row wire) ----
            Ktl = pB.tile([128, HP, QPC], f8, tag="Ktl", bufs=1)
            for hp in range(HP):
                ps = pBp.tile([128, QPC], f32, tag="ps")
                for ci in range(NCH):
                    nc.tensor.matmul(ps[:], wk_all[:, ci, hp, :],
                                     snT[:, ci, :],
                                     start=(ci == 0), stop=(ci == NCH - 1))
                nc.vector.tensor_copy(Ktl[:, hp, :], ps[:])
            nc.sync.dma_start(
                out=kc_in[0:48, :],
                in_=Ktl[0:48].rearrange("p a b -> p (a b)"))
            nc.sync.dma_start(
                out=kc_in[48:96, :],
                in_=Ktl[64:112].rearrange("p a b -> p (a b)"))
            nc.gpsimd.collective_compute(
                "AllGather", mybir.AluOpType.bypass,
                replica_groups=[[0, 1, 2, 3], [4, 5, 6, 7]],
                ins=[kc_in[:]], outs=[kc_out[:]])
            # unpack gathered K per head
            # (even heads ride wire rows 0..47, odd heads rows 48..95)
            for r in range(4):
                nc.sync.dma_start(
                    out=Kt128[0:48, 0 : H : 2, ts(r * QPC, QPC)],
                    in_=kc_out[r][0:48].rearrange("p (a b) -> p a b", a=HP))
                nc.sync.dma_start(
                    out=Kt128[0:48, 1 : H : 2, ts(r * QPC, QPC)],
                    in_=kc_out[r][48:96].rearrange("p (a b) -> p a b", a=HP))

            # ---- V projection into the ones-augmented layout, V AllGather --
            Vl49 = pB.tile([128, QT, H, VW], f8, tag="Vl49", bufs=1)
            nc.vector.memset(Vl49[:, :, :, DH : DH + 1], 1.0)
            for tt in range(QT):
                for cg in range(2):
                    psv = pBp.tile([128, 384], f32, tag="psv")
                    for ci in range(NCH):
                        nc.tensor.matmul(psv[:], snT[:, ci, ts(tt * 128, 128)],
                                         wv_all[:, ci, ts(cg * 3, 3)],
                                         start=(ci == 0), stop=(ci == NCH - 1))
                    nc.vector.tensor_copy(
                        Vl49[:, tt, ts(cg * 8, 8), 0:DH],
                        psv[:].rearrange("p (h d) -> p h d", h=8))
            nc.sync.dma_start(out=vc_in[:],
                              in_=Vl49[:].rearrange("p a h w -> p (a h w)"))
            nc.gpsimd.collective_compute(
                "AllGather", mybir.AluOpType.bypass,
                replica_groups=[[0, 1, 2, 3], [4, 5, 6, 7]],
                ins=[vc_in[:]], outs=[vc_out[:]])
            for r in range(4):
                nc.sync.dma_start(
                    out=Vf8[:, ts(r * QT, QT), :, :],
                    in_=vc_out[r].rearrange("p (a h w) -> p a h w",
                                            a=QT, h=H))
                nc.vector.tensor_copy(V49g[:, ts(r * QT, QT), :, :],
                                      Vf8[:, ts(r * QT, QT), :, :])

            # ---- Q projection (pair-packed psum -> per-head 128-row tiles) --
            Qt = pB.tile([128, HP, QPC], f8, tag="Qt", bufs=1)
            for hp in range(HP):
                wc = pB.tile([128, NCH, 128], bf16, tag="wc")
                nc.sync.dma_start(out=wc[:], in_=dram["wq"][:, hp, :, :])
                ps = pBp.tile([128, QPC], f32, tag="ps")
                for ci in range(NCH):
                    nc.tensor.matmul(ps[:], wc[:, ci, :], snT[:, ci, :],
                                     start=(ci == 0), stop=(ci == NCH - 1))
                nc.vector.tensor_copy(Qt[:, hp, :], ps[:])
            nc.sync.dma_start(
                out=Qt128[0:48, 0 : H : 2, :],
                in_=Qt[0:48, :, :])
            nc.sync.dma_start(
                out=Qt128[0:48, 1 : H : 2, :],
                in_=Qt[64:112, :, :])

            # ---- G gate ----
            wg_all = pBw.tile([128, NCH, NCH, 128], bf16, tag="w6")
            nc.sync.dma_start(out=wg_all[:], in_=dram["wg"][:])
            for co in range(NCH):
                psgf = pBp.tile([128, QPC], f32, tag="psgf")
                for ci in range(NCH):
                    nc.tensor.matmul(psgf[:], wg_all[:, ci, co, :],
                                     snT[:, ci, :],
                                     start=(ci == 0), stop=(ci == NCH - 1))
                nc.scalar.activation(sig_g[:, co, :], psgf[:], AF.Sigmoid)

            # ---- precompute g1 / g2 gates (cond-only) ----
            # schedule this block into the AllGather wait window (it is only
            # needed by phases D/E, ~200us later)
            precomp = contextlib.ExitStack()
            precomp.enter_context(tc.tile_wait_until(0.095))
            g1_all = pBw.tile([128, NCH, NCH, 128], bf16, tag="w6")
            nc.sync.dma_start(out=g1_all[:], in_=dram["g1w"][:])
            for co in range(NCH):
                ps1 = pBp.tile([128, QPC], f32, tag="ps")
                for ci in range(NCH):
                    nc.tensor.matmul(ps1[:], g1_all[:, ci, co, :],
                                     cT[:, ci, :],
                                     start=(ci == 0), stop=(ci == NCH - 1))
                nc.scalar.activation(sig1[:, co, :], ps1[:], AF.Sigmoid,
                                     bias=cvec["g1b"][:, co, :])
            g2_all = pBw.tile([128, NCH, NCH, 128], bf16, tag="w6")
            nc.sync.dma_start(out=g2_all[:], in_=dram["g2w"][:])
            for co in range(NCH):
                ps2 = pBp.tile([128, QPC], f32, tag="ps")
                for ci in range(NCH):
                    nc.tensor.matmul(ps2[:], g2_all[:, ci, co, :],
                                     cT[:, ci, :],
                                     start=(ci == 0), stop=(ci == NCH - 1))
                nc.scalar.activation(sig2[:, co, :], ps2[:], AF.Sigmoid,
                                     bias=cvec["g2b"][:, co, :])

            # ---- precompute AdaLN2 cond-side matmuls ----
            a2gw_all = pBw.tile([128, NCH, NCH, 128], bf16, tag="w6")
            nc.sync.dma_start(out=a2gw_all[:], in_=dram["a2gw"][:])
            for co in range(NCH):
                psg = pBp.tile([128, QPC], f32, tag="ps")
                for ci in range(NCH):
                    nc.tensor.matmul(psg[:], a2gw_all[:, ci, co, :],
                                     cn[:, ci, :],
                                     start=(ci == 0), stop=(ci == NCH - 1))
                nc.scalar.copy(psG2sb[:, co, :], psg[:])
            a2bw_all = pBw.tile([128, NCH, NCH, 128], bf16, tag="w6")
            nc.sync.dma_start(out=a2bw_all[:], in_=dram["a2bw"][:])
            for co in range(NCH):
                psb = pBp.tile([128, QPC], f32, tag="ps")
                for ci in range(NCH):
                    nc.tensor.matmul(psb[:], a2bw_all[:, ci, co, :],
                                     cn[:, ci, :],
                                     start=(ci == 0), stop=(ci == NCH - 1))
                nc.scalar.copy(psB2sb[:, co, :], psb[:])
            # premultiply the two phase-D gates off the critical path
            for co in range(NCH):
                nc.vector.tensor_mul(gate12[:, co, :], sig1[:, co, :],
                                     sig_g[:, co, :])
            precomp.close()

        snstack.close()  # free snT

        # ==================================================================
        # Phase C: attention (S^T layout) -> att_nT
        # ==================================================================
        dstack = contextlib.ExitStack()
        pDw = dstack.enter_context(tc.tile_pool(name="pDw", bufs=1))
        att_nT = pAtt.tile([128, HP, QPC], bf16, tag="att_nT")
        attU = pAtt.tile([128, HP, QPC], bf16, tag="attU")
        nc.gpsimd.memset(attU[:], 0.0)
        # prefetch phase-D operands ahead of the E-table DMA stream
        wo_all = pDw.tile([128, HP, NCH, 128], bf16, tag="wo_all")
        nc.sync.dma_start(out=wo_all[:], in_=dram["wo"][:])
        sqT = pDw.tile([128, NCH, QPC], f32, tag="sqT")
        nc.sync.dma_start(out=sqT[:], in_=dram["sqT"][:])
        Dstage = pAtt.tile([128, HP, QPC], bf16, tag="Dstage")
        Dall = pAtt.tile([16, QPC], bf16, tag="Dall")
        Dinv = pAtt.tile([16, QPC], bf16, tag="Dinv")

        # per-head key-block groups of 4/2 blocks, alternating through two
        # PSUM pools (4 + 2 banks) so consecutive groups' matmuls stay
        # decoupled from the exp drain (deep PE/ACT pipelining).
        # first group needs only rank 0's K blocks so the S/exp stream can
        # start as soon as the first gather unpack lands
        GRP = [(0, 2), (2, 4), (6, 2), (8, 4)]
        with tc.tile_pool(name="pEt", bufs=7) as pEt, \
             tc.tile_pool(name="pPt", bufs=3) as pPt, \
             tc.tile_pool(name="pP2", bufs=5) as pP2, \
             tc.tile_pool(name="psS4", bufs=1, space="PSUM") as psS4, \
             tc.tile_pool(name="psS2", bufs=1, space="PSUM") as psS2, \
             tc.tile_pool(name="psPV", bufs=2, space="PSUM") as psPVp:
            for hp in range(HP):
                psPV = psPVp.tile([128, QPC], f32, tag="pv", name="pv")
                for s in range(2):
                    h, plo = 2 * hp + s, DHP * s
                    for gi, (kb0, nkb) in enumerate(GRP):
                        Et = pEt.tile([128, 4 * QPC], bf16, tag="Et")
                        Etv = Et[:, 0 : nkb * QPC]
                        nc.sync.dma_start(
                            out=Etv.rearrange("p (a b) -> p a b", b=QPC),
                            in_=dram["E"][h][:, ts(kb0, nkb), :])
                        pool = psS4 if nkb == 4 else psS2
                        psS = pool.tile([128, nkb * 512], f32,
                                        tag="sg", name="sg")
                        for j in range(nkb):
                            kb = kb0 + j
                            nc.tensor.matmul(
                                psS[:, ts(j * 512, QPC)],
                                Kt128[:, h, ts(kb * 128, 128)],
                                Qt128[:, h, :],
                                start=True, stop=True)
                        Pt = pPt.tile([128, 4 * QPC], bf16, tag="Pt")
                        nc.scalar.activation(
                            Pt[:, 0 : nkb * QPC].rearrange(
                                "p (a b) -> p a b", b=QPC),
                            psS[:].rearrange("p (a b) -> p a b", b=512)
                               [:, :, 0:QPC],
                            AF.Exp)
                        P2 = pP2.tile([128, 4 * QPC], bf16, tag="P2")
                        # small groups' multiplies go to the idle GpSimd
                        mul_eng = nc.vector if nkb == 4 else nc.gpsimd
                        mul_eng.tensor_mul(P2[:, 0 : nkb * QPC],
                                           Pt[:, 0 : nkb * QPC], Etv)
                        P2v = P2[:, 0 : nkb * QPC].rearrange(
                            "p (a b) -> p a b", b=QPC)
                        for j in range(nkb):
                            kb = kb0 + j
                            nc.tensor.matmul(
                                psPV[plo : plo + VW, :],
                                V49g[:, kb, h, :],
                                P2v[:, j, :],
                                start=(gi == 0 and j == 0),
                                stop=(gi == len(GRP) - 1 and j == nkb - 1),
                                tile_position=(0, plo),
                                skip_group_check=True)
                # drain denominators + unnormalized attention
                # (DVE base partition must be 32-aligned; rows 32..48 and
                #  96..112 are all PV-written, den sits at 48 / 112)
                nc.vector.tensor_copy(Dstage[32:49, hp, :], psPV[32:49, :])
                nc.vector.tensor_copy(Dstage[96:113, hp, :],
                                      psPV[96:113, :])
                for s in range(2):
                    plo = DHP * s
                    nc.vector.tensor_copy(attU[plo : plo + DH, hp, :],
                                          psPV[plo : plo + DH, :])
            # tail: batched reciprocal + per-pair broadcast + normalize
            nc.sync.dma_start(out=Dall[0:8, :], in_=Dstage[48:49, :, :])
            nc.sync.dma_start(out=Dall[8:16, :], in_=Dstage[112:113, :, :])
            with nc.allow_low_precision(reason="bf16 softmax denominators"):
                nc.vector.reciprocal(Dinv[:], Dall[:])
            for hp in range(HP):
                psb = psPVp.tile([128, QPC], f32, tag="pv", name="db")
                nc.tensor.matmul(psb[:], selm_sb[:, hp, :], Dinv[:],
                                 start=True, stop=True)
                nc.vector.tensor_mul(att_nT[:, hp, :], attU[:, hp, :],
                                     psb[:])

        # ==================================================================
        # Phase D: wo + gates + residual -> s_new
        # ==================================================================
        sn2 = outer.tile([128, NCH, QPC], f8, tag="sn2")
        xb2 = outer.tile([128, NCH, QPC], bf16, tag="xb2")
        sq2 = outer.tile([128, NCH, QPC], bf16, tag="sq2")
        with tc.tile_pool(name="pD", bufs=2) as pD, \
             tc.tile_pool(name="pDp", bufs=2, space="PSUM") as pDp:
            for co in range(NCH):
                pso = pDp.tile([128, QPC], f32, tag="pso")
                for ci in range(HP):
                    nc.tensor.matmul(pso[:], wo_all[:, ci, co, :],
                                     att_nT[:, ci, :],
                                     start=(ci == 0), stop=(ci == HP - 1))
                t2 = pD.tile([128, QPC], bf16, tag="t2")
                nc.vector.tensor_mul(t2[:], gate12[:, co, :], pso[:])
                nc.vector.tensor_add(s_new[:, co, :], sqT[:, co, :], t2[:])
                # feed the AdaLN2 stats incrementally
                nc.vector.tensor_copy(xb2[:, co, :], s_new[:, co, :])
                nc.vector.tensor_mul(sq2[:, co, :], xb2[:, co, :],
                                     xb2[:, co, :])

        dstack.close()   # free wo_all/sqT
        attstack.close()  # free snT/Kt/Qt/V49g/sig_g/sig1/att tiles

        # ==================================================================
        # Phase E: AdaLN2 (combine with precomputed cond matmuls) -> sn2
        # ==================================================================
        with tc.tile_pool(name="pE", bufs=1) as pE, \
             tc.tile_pool(name="pEt2", bufs=3) as pEt2, \
             tc.tile_pool(name="pEp", bufs=2, space="PSUM") as pEp:
            xn2 = pE.tile([128, NCH, QPC], bf16, tag="xn2")
            Rs2 = pE.tile([128, QPC], f32, tag="Rs2")
            Mb2 = pE.tile([128, QPC], f32, tag="Mb2")
            ln_stats(xb2, Mb2, Rs2, "s2", sq_pre=sq2)
            with tc.tile_pool(name="bcEw", bufs=3) as bw2:
                ln_apply(s_new, Mb2, Rs2, xn2, bw2)
            gb, bb = cvec["a2gb"], cvec["a2bb"]
            for co in range(NCH):
                sig = pEt2.tile([128, QPC], bf16, tag="sig")
                nc.scalar.activation(sig[:], psG2sb[:, co, :], AF.Sigmoid,
                                     bias=gb[:, co, :])
                t1 = pEt2.tile([128, QPC], bf16, tag="t1")
                nc.vector.tensor_mul(t1[:], sig[:], xn2[:, co, :])
                nc.vector.scalar_tensor_tensor(
                    sn2[:, co, :], psB2sb[:, co, :], bb[:, co, :],
                    t1[:], op0=OP.add, op1=OP.add)

        # ==================================================================
        # Phase F: SwiGLU + g2 gate + residual -> outT
        # ==================================================================
        with tc.tile_pool(name="pF", bufs=3) as pF, \
             tc.tile_pool(name="pFh", bufs=1) as pFh, \
             tc.tile_pool(name="pFp", bufs=2, space="PSUM") as pFp:
            DR = mybir.MatmulPerfMode.DoubleRow
            hT = pFh.tile([128, FCH, QPC], f8, tag="hT")
            for co in range(FCH):
                gwc = pF.tile([128, NCH, 128], f8, tag="gwc")
                nc.sync.dma_start(out=gwc[:], in_=dram["swg"][:, co, :, :])
                uwc = pF.tile([128, NCH, 128], f8, tag="uwc")
                nc.sync.dma_start(out=uwc[:], in_=dram["swu"][:, co, :, :])
                psG = pFp.tile([128, QPC], f32, tag="psG")
                psU = pFp.tile([128, QPC], f32, tag="psU")
                for c in range(NCH // 2):
                    nc.tensor.matmul(psG[:], gwc[:, ts(2 * c, 2), :],
                                     sn2[:, ts(2 * c, 2), :],
                                     start=(c == 0), stop=(c == NCH // 2 - 1),
                                     perf_mode=DR)
                    nc.tensor.matmul(psU[:], uwc[:, ts(2 * c, 2), :],
                                     sn2[:, ts(2 * c, 2), :],
                                     start=(c == 0), stop=(c == NCH // 2 - 1),
                                     perf_mode=DR)
                sg = pF.tile([128, QPC], bf16, tag="sg")
                nc.scalar.activation(sg[:], psG[:], AF.Sigmoid, scale=1.0 / 128)
                tg = pF.tile([128, QPC], bf16, tag="tg")
                nc.vector.scalar_tensor_tensor(
                    tg[:], psG[:], 1.0 / 128, sg[:],
                    op0=OP.mult, op1=OP.mult)
                nc.vector.scalar_tensor_tensor(
                    hT[:, co, :], psU[:], 1.0 / 128, tg[:],
                    op0=OP.mult, op1=OP.mult)
            outT = pFh.tile([128, NCH, QPC], f32, tag="outT")
            for co in range(NCH):
                dwc = pF.tile([128, FCH, 128], f8, tag="dwc")
                nc.sync.dma_start(out=dwc[:], in_=dram["swd"][:, co, :, :])
                psD = pFp.tile([128, QPC], f32, tag="psD")
                for c in range(FCH // 2):
                    nc.tensor.matmul(psD[:], dwc[:, ts(2 * c, 2), :],
                                     hT[:, ts(2 * c, 2), :],
                                     start=(c == 0), stop=(c == FCH // 2 - 1),
                                     perf_mode=DR)
                t3 = pF.tile([128, QPC], bf16, tag="t3")
                nc.vector.scalar_tensor_tensor(
                    t3[:], psD[:], 1.0 / 128, sig2[:, co, :],
                    op0=OP.mult, op1=OP.mult)
                nc.vector.tensor_add(outT[:, co, :], s_new[:, co, :], t3[:])
            nc.sync.dma_start(out=dram["outT"][:], in_=outT[:])


# ----------------------------------------------------------------------------
# public entry point
# ----------------------------------------------------------------------------

def get_program():
    if "nc" not in _PROGRAM_CACHE:
        _PROGRAM_CACHE["nc"] = build_program()
    return _PROGRAM_CACHE["nc"]


def kernel(**inputs):
    from concourse.bass_utils import run_bass_kernel_spmd

    nc = get_program()
    in_maps = host_prep(inputs)
    res = run_bass_kernel_spmd(nc, in_maps, list(range(NCORES)))
    return assemble_output(res.results)


if __name__ == "__main__":
    import reference

    inputs = {k: np.asarray(v) for k, v in reference.setup_inputs().items()}
    out = kernel(**inputs)
    print("kernel output", out.shape, out.dtype)


# revision 102
# speedup vs baseline: 1.0680x; 1.0680x over previous
"""Trainium2 Bass kernel for nn_DiffusionModuleV2 (dense transformer block).

Sharding: 8 cores = 2 batches x 4 query-quarters; fully token-parallel
(AdaLN, projections, FFN on the core's own 384 tokens) with AllGathers
per 4-core batch group for K/V.

Device layout: transposed activations [D-partitions (6x128 chunks), token-free].
Attention is computed in the S^T layout: S^T[k, q] = K_kb^T @ Q per 128-key
block, so the softmaxed P^T feeds P@V directly as the moving operand (no
transposes).  The positional bias is applied multiplicatively
(P = exp(S) * exp(bias), exp(bias) gathered on host), and the softmax
denominator comes free from a ones-column appended to V.  Normalization is
batched at the attention tail (one reciprocal + per-head-pair PE broadcast).
g1/g2 gates and the AdaLN2 cond-side matmuls are precomputed during the
collective window to hide the AllGather latency.
"""

import sys

sys.path.insert(0, "/opt/trn_rl_repo")

import numpy as np
import ml_dtypes

BF = ml_dtypes.bfloat16
F8 = ml_dtypes.float8_e4m3
F32 = np.float32

B, N, D, H = 2, 1536, 768, 16
DH, DHP = 48, 64
FF = 4 * D
EPS = 1e-5
NCORES = 8
QPC = N // 4          # 384 queries per core
NCH = D // 128        # 6
FCH = FF // 128       # 24
HP = H // 2           # 8 head pairs
NKB = N // 128        # 12 key blocks of 128
QT = QPC // 128       # 3 token tiles of 128
VW = DH + 1           # 49: V columns + ones column per head

_PROGRAM_CACHE = {}


def ts(start, size):
    return slice(start, start + size)


# ----------------------------------------------------------------------------
# host-side layout helpers
# ----------------------------------------------------------------------------

def _chunkT(x_t):  # (D, T) -> [128, NCH, T]
    d, t = x_t.shape
    return np.ascontiguousarray(x_t.reshape(d // 128, 128, t).transpose(1, 0, 2))


def _wtiles(w):  # (Din, Cout) -> [128, Din/128, Cout/128, 128]
    din, cout = w.shape
    return np.ascontiguousarray(
        w.reshape(din // 128, 128, cout // 128, 128).transpose(1, 0, 2, 3)
    )


def _wtilesT(w):  # (Din, Cout) -> [128, Cout/128, Din/128, 128]  (co-major)
    din, cout = w.shape
    return np.ascontiguousarray(
        w.reshape(din // 128, 128, cout // 128, 128).transpose(1, 2, 0, 3)
    )


def _colvec(v):  # (D,) per-out-col bias -> [128, NCH, 1]
    return np.ascontiguousarray(v.reshape(NCH, 128, 1).transpose(1, 0, 2)).astype(F32)


def _rowvec(v):  # (D,) -> [1, NCH, 128]  (K=1 matmul lhsT slices)
    return np.ascontiguousarray(v.reshape(1, NCH, 128)).astype(F32)


def _pad_qk(w):  # (D, H*48) -> (D, H*64), head h cols at 64h..64h+47
    out = np.zeros((D, H * DHP), w.dtype)
    for h in range(H):
        out[:, h * DHP : h * DHP + DH] = w[:, h * DH : (h + 1) * DH]
    return out


def _pad_wo(w):  # (H*48, D) -> (H*64, D), head h rows at 64h..64h+47
    out = np.zeros((H * DHP, D), w.dtype)
    for h in range(H):
        out[h * DHP : h * DHP + DH, :] = w[h * DH : (h + 1) * DH, :]
    return out


def prep_weights(inputs):
    w = {}
    f = lambda k: np.asarray(inputs[k], np.float64)

    def adaln(pfx, ln_w, ln_b, gw, gb, bw):
        gw_eff = (ln_w[:, None] * gw).astype(BF)
        bw_eff = (ln_w[:, None] * bw).astype(BF)
        w[pfx + "gw"] = _wtiles(gw_eff)
        w[pfx + "bw"] = _wtiles(bw_eff)
        w[pfx + "gb"] = _colvec(gb + ln_b @ gw)
        w[pfx + "bb"] = _colvec(ln_b @ bw)
        w[pfx + "csg"] = _rowvec(-gw_eff.astype(np.float64).sum(0))
        w[pfx + "csb"] = _rowvec(-bw_eff.astype(np.float64).sum(0))

    adaln("a1", f("a1_ln_w"), f("a1_ln_b"), f("a1_gw"), f("a1_gb"), f("a1_bw"))
    adaln("a2", f("a2_ln_w"), f("a2_ln_b"), f("a2_gw"), f("a2_gb"), f("a2_bw"))

    # split the 1/sqrt(DH) between Q and K so both land in fp8's sweet spot
    w["wq"] = _wtilesT(_pad_qk((f("wq") * DH**-0.25).astype(BF)))
    w["wk"] = _wtiles(_pad_qk((f("wk") * DH**-0.25).astype(BF)))
    w["wv"] = _wtiles(f("wv").astype(BF))
    w["wg"] = _wtiles(f("wg").astype(BF))
    w["wo"] = _wtiles(_pad_wo(f("wo").astype(BF)))
    w["g1w"] = _wtiles(f("g1_w").astype(BF))
    w["g1b"] = _colvec(f("g1_b"))
    w["g2w"] = _wtiles(f("g2_w").astype(BF))
    w["g2b"] = _colvec(f("g2_b"))
    # SwiGLU weights in fp8 (DoubleRow), scaled x128 into e4m3's range;
    # the 1/128 is folded into the activation/gating ops on device.
    # co-major layout so the per-co streamed DMA slices are contiguous.
    w["swg"] = _wtilesT((f("sw_gate") * 128.0).astype(F8))
    w["swu"] = _wtilesT((f("sw_up") * 128.0).astype(F8))
    w["swd"] = _wtilesT((f("sw_down") * 128.0).astype(F8))

    # den-broadcast selectors: Dall row (hp) -> out partitions 0..47,
    # row (8+hp) -> out partitions 64..111
    selm = np.zeros((16, HP, 128), BF)
    for hp in range(HP):
        selm[hp, hp, 0:DH] = 1.0
        selm[8 + hp, hp, DHP : DHP + DH] = 1.0
    w["selm"] = selm
    return w


def host_prep(inputs):
    """Build the 8 per-core input maps (numpy, dtypes matching DRAM decls)."""
    wts = prep_weights(inputs)
    s = np.asarray(inputs["s"], F32)
    cond = np.asarray(inputs["s_cond"], F32)
    pw = np.asarray(inputs["pos_weight"], np.float64)  # (H, NBINS)
    expw = np.exp(pw).astype(F32)
    bins = np.asarray(inputs["pos_bins"])

    in_maps = []
    for c in range(NCORES):
        b, qi = c // 4, c % 4
        qsl = slice(qi * QPC, (qi + 1) * QPC)
        m = dict(wts)
        m["sT"] = _chunkT(s[b].T[:, qsl]).astype(BF)
        m["cT"] = _chunkT(cond[b].T[:, qsl]).astype(BF)
        m["sqT"] = _chunkT(s[b].T[:, qsl]).astype(F32)
        # E[h, k, kb, q] = exp(pw[h, bins[b, q, kb*128+k]])  (key-transposed)
        binsT = bins[b, qsl].T                    # (N keys, QPC queries)
        arr = expw[:, binsT]                      # (H, N, QPC)
        arr = arr.reshape(H, NKB, 128, QPC).transpose(0, 2, 1, 3)
        m["E"] = np.ascontiguousarray(arr.astype(BF))  # (H, 128, NKB, QPC)
        in_maps.append(m)
    return in_maps


def assemble_output(results):
    out = np.empty((B, N, D), F32)
    for c in range(NCORES):
        b, qi = c // 4, c % 4
        t = np.asarray(results[c]["outT"])  # [128, NCH, QPC]
        out[b, qi * QPC : (qi + 1) * QPC, :] = (
            t.transpose(1, 0, 2).reshape(D, QPC).T)
    return out


# ----------------------------------------------------------------------------
# device program
# ----------------------------------------------------------------------------

def declare_io(nc, mybir):
    f32, bf16 = mybir.dt.float32, mybir.dt.bfloat16
    dram = {}

    def din(name, shape, dt):
        dram[name] = nc.dram_tensor(name, shape, dt, kind="ExternalInput")

    din("sT", [128, NCH, QPC], bf16)
    din("cT", [128, NCH, QPC], bf16)
    din("sqT", [128, NCH, QPC], f32)
    din("E", [H, 128, NKB, QPC], bf16)
    din("selm", [16, HP, 128], bf16)
    for pfx in ("a1", "a2"):
        din(pfx + "gw", [128, NCH, NCH, 128], bf16)
        din(pfx + "bw", [128, NCH, NCH, 128], bf16)
        din(pfx + "gb", [128, NCH, 1], f32)
        din(pfx + "bb", [128, NCH, 1], f32)
        din(pfx + "csg", [1, NCH, 128], f32)
        din(pfx + "csb", [1, NCH, 128], f32)
    din("wq", [128, HP, NCH, 128], bf16)
    din("wk", [128, NCH, HP, 128], bf16)
    din("wv", [128, NCH, NCH, 128], bf16)
    din("wg", [128, NCH, NCH, 128], bf16)
    din("wo", [128, HP, NCH, 128], bf16)
    din("g1w", [128, NCH, NCH, 128], bf16)
    din("g1b", [128, NCH, 1], f32)
    din("g2w", [128, NCH, NCH, 128], bf16)
    din("g2b", [128, NCH, 1], f32)
    din("swg", [128, FCH, NCH, 128], mybir.dt.float8e4)
    din("swu", [128, FCH, NCH, 128], mybir.dt.float8e4)
    din("swd", [128, NCH, FCH, 128], mybir.dt.float8e4)
    dram["outT"] = nc.dram_tensor("outT", [128, NCH, QPC], f32,
                                  kind="ExternalOutput")
    return dram


def build_program():
    import concourse.mybir as mybir
    import concourse.tile as tile
    from concourse import bacc

    nc = bacc.Bacc("TRN2", target_bir_lowering=False, debug=False,
                   num_devices=NCORES)
    dram = declare_io(nc, mybir)
    with tile.TileContext(nc) as tc:
        _emit(nc, tc, dram, mybir)
    nc.compile()
    return nc


def _emit(nc, tc, dram, mybir):
    import contextlib

    from concourse.bass_isa import ReduceOp

    f32, bf16 = mybir.dt.float32, mybir.dt.bfloat16
    f8 = mybir.dt.float8e4
    AF = mybir.ActivationFunctionType
    OP = mybir.AluOpType

    ctx = contextlib.ExitStack()
    with ctx:
        const = ctx.enter_context(tc.tile_pool(name="const", bufs=1))
        outer = ctx.enter_context(tc.tile_pool(name="outer", bufs=1))

        # ---- constants / small residents ----
        onesmat = const.tile([128, 128], bf16, tag="onesmat")
        nc.vector.memset(onesmat[:], 1.0)
        cvec = {}
        for name in ("a1gb", "a1bb", "a2gb", "a2bb", "g1b", "g2b"):
            t = const.tile(list(dram[name].shape), dram[name].dtype,
                           name="c_" + name, tag=name)
            nc.sync.dma_start(out=t[:], in_=dram[name][:])
            cvec[name] = t

        selm_sb = const.tile([16, HP, 128], bf16, tag="selm")
        nc.sync.dma_start(out=selm_sb[:], in_=dram["selm"][:])
        eps128 = const.tile([128, 1], f32, tag="eps128")
        nc.vector.memset(eps128[:], EPS)

        # ---- persistent activations ----
        cT = outer.tile([128, NCH, QPC], bf16, tag="cT")
        for ci in range(NCH):
            nc.sync.dma_start(out=cT[:, ci, :], in_=dram["cT"][:, ci, :])
        s_new = outer.tile([128, NCH, QPC], f32, tag="s_new")
        Rs_c = outer.tile([128, QPC], f32, tag="Rs_c")

        # ------------------------------------------------------------------
        def ln_stats(x_bf, Mb, Rb, tag, sq_pre=None):
            """LN stats over the partition (D) axis via all-ones matmuls:
            ones.T @ x sums the partitions AND broadcasts the result to all
            128 rows in one full-activity PE instruction per chunk."""
            with tc.tile_pool(name="st_" + tag, bufs=1) as wp, \
                 tc.tile_pool(name="stp_" + tag, bufs=1, space="PSUM") as pp:
                psx = pp.tile([128, QPC], f32, tag="psx")
                pss = pp.tile([128, QPC], f32, tag="pss")
                for ci in range(NCH):
                    nc.tensor.matmul(psx[:], onesmat[:], x_bf[:, ci, :],
                                     start=(ci == 0), stop=(ci == NCH - 1))
                for ci in range(NCH):
                    if sq_pre is not None:
                        sq = sq_pre[:, ci, :]
                    else:
                        sqt = wp.tile([128, QPC], bf16, tag="sq", bufs=3)
                        nc.vector.tensor_mul(sqt[:], x_bf[:, ci, :],
                                             x_bf[:, ci, :])
                        sq = sqt[:]
                    nc.tensor.matmul(pss[:], onesmat[:], sq,
                                     start=(ci == 0), stop=(ci == NCH - 1))
                nc.vector.tensor_scalar_mul(Mb[:], psx[:], 1.0 / D)
                msq = wp.tile([128, QPC], f32, tag="msq")
                nc.vector.tensor_mul(msq[:], Mb[:], Mb[:])
                v = wp.tile([128, QPC], f32, tag="v")
                nc.vector.scalar_tensor_tensor(
                    v[:], pss[:], 1.0 / D, msq[:],
                    op0=OP.mult, op1=OP.subtract)
                lnv = wp.tile([128, QPC], f32, tag="lnv")
                nc.scalar.activation(lnv[:], v[:], AF.Ln, bias=eps128[:])
                nc.scalar.activation(Rb[:], lnv[:], AF.Exp, scale=-0.5)

        def ln_apply(x_bf, Mb, R_sb, xn, wp):
            """xn = (x - Mb) * R, with Mb/R already broadcast [128, T]."""
            for ch in range(NCH):
                d = wp.tile([128, QPC], f32, tag="d")
                nc.vector.tensor_sub(d[:], x_bf[:, ch, :], Mb[:])
                nc.vector.tensor_mul(xn[:, ch, :], d[:], R_sb[:])

        def adaln_gb(pfx, cn_t, xn, sn_out, gw_all, bw_all):
            """sn = sigmoid(psG + gb) * xn + (psB + bb), where
            psG/psB = W^T @ cn and cn = LN(cond) (scale folded into cn)."""
            gb, bb = cvec[pfx + "gb"], cvec[pfx + "bb"]
            with tc.tile_pool(name=pfx + "t", bufs=3) as tp, \
                 tc.tile_pool(name=pfx + "p", bufs=2, space="PSUM") as pp:
                for co in range(NCH):
                    gwc, bwc = gw_all[:, :, co, :], bw_all[:, :, co, :]
                    psg = pp.tile([128, QPC], f32, tag="psg")
                    psb = pp.tile([128, QPC], f32, tag="psb")
                    for ci in range(NCH):
                        nc.tensor.matmul(psg[:], gwc[:, ci, :],
                                         cn_t[:, ci, :],
                                         start=(ci == 0), stop=(ci == NCH - 1))
                        nc.tensor.matmul(psb[:], bwc[:, ci, :],
                                         cn_t[:, ci, :],
                                         start=(ci == 0), stop=(ci == NCH - 1))
                    sig = tp.tile([128, QPC], bf16, tag="sig")
                    nc.scalar.activation(sig[:], psg[:], AF.Sigmoid,
                                         bias=gb[:, co, :])
                    t1 = tp.tile([128, QPC], bf16, tag="t1")
                    nc.vector.tensor_mul(t1[:], sig[:], xn[:, co, :])
                    nc.vector.scalar_tensor_tensor(
                        sn_out[:, co, :], psb[:], bb[:, co, :],
                        t1[:], op0=OP.add, op1=OP.add)

        # ==================================================================
        # Phase A: AdaLN1 -> snT
        # ==================================================================
        attstack = contextlib.ExitStack()
        pAtt = attstack.enter_context(tc.tile_pool(name="pAtt", bufs=1))
        dp = attstack.enter_context(
            tc.tile_pool(name="ccd", bufs=1, space="DRAM"))
        cn = pAtt.tile([128, NCH, QPC], bf16, tag="cn")
        # K/Q zero-padded to full 128-row contraction: the wasted MACs are
        # free (matmul time is N-streaming bound) and the full-activity
        # matmuls keep the PE clock un-throttled.
        Kt128 = pAtt.tile([128, H, N], f8, tag="Kt128")
        Qt128 = pAtt.tile([128, H, QPC], f8, tag="Qt128")
        snstack = contextlib.ExitStack()
        pSn = snstack.enter_context(tc.tile_pool(name="pSn", bufs=1))
        snT = pSn.tile([128, NCH, QPC], bf16, tag="snT")
        # zero the attention pad rows early, off the DVE (GpSimd is idle here)
        nc.gpsimd.memset(Kt128[:], 0.0)
        nc.gpsimd.memset(Qt128[:], 0.0)
        with tc.tile_pool(name="pA", bufs=1) as pA:
            sT = pA.tile([128, NCH, QPC], bf16, tag="sT")
            for ci in range(NCH):
                nc.sync.dma_start(out=sT[:, ci, :], in_=dram["sT"][:, ci, :])
            a1gw_all = pA.tile([128, NCH, NCH, 128], bf16, tag="a1gw_all")
            nc.sync.dma_start(out=a1gw_all[:], in_=dram["a1gw"][:])
            a1bw_all = pA.tile([128, NCH, NCH, 128], bf16, tag="a1bw_all")
            nc.sync.dma_start(out=a1bw_all[:], in_=dram["a1bw"][:])
            xn = pA.tile([128, NCH, QPC], bf16, tag="xn")
            Rs_s = pA.tile([128, QPC], f32, tag="Rs_s")
            Mb_c = pA.tile([128, QPC], f32, tag="Mb_c")
            Mb_s = pA.tile([128, QPC], f32, tag="Mb_s")
            ln_stats(cT, Mb_c, Rs_c, "c")
            ln_stats(sT, Mb_s, Rs_s, "s")
            # normalized cond (LN sans affine; affine folded into weights),
            # reused by AdaLN1 + the AdaLN2 precompute
            with tc.tile_pool(name="bcAw", bufs=3) as bw:
                ln_apply(cT, Mb_c, Rs_c, cn, bw)
                ln_apply(sT, Mb_s, Rs_s, xn, bw)
            adaln_gb("a1", cn, xn, snT,
                     gw_all=a1gw_all, bw_all=a1bw_all)

        # ==================================================================
        # Phase B: projections + K/V AllGather + gate precompute
        # ==================================================================
        V49g = pAtt.tile([128, NKB, H, VW], bf16, tag="V49g")
        sig_g = pAtt.tile([128, NCH, QPC], bf16, tag="sig_g")
        sig1 = pAtt.tile([128, NCH, QPC], bf16, tag="sig1")
        gate12 = pAtt.tile([128, NCH, QPC], bf16, tag="gate12")
        sig2 = outer.tile([128, NCH, QPC], bf16, tag="sig2")
        psG2sb = outer.tile([128, NCH, QPC], bf16, tag="psG2sb")
        psB2sb = outer.tile([128, NCH, QPC], bf16, tag="psB2sb")

        with tc.tile_pool(name="pB", bufs=2) as pB, \
             tc.tile_pool(name="pBw", bufs=2) as pBw, \
             tc.tile_pool(name="pBp", bufs=2, space="PSUM") as pBp:
            KB = HP * QPC              # 3072
            VB = QT * H * VW           # 2352
            kc_in = dp.tile([96, KB], f8, name="kc_in")
            kc_out = dp.tile([4, 96, KB], f8, name="kc_out")
            vc_in = dp.tile([128, VB], f8, name="vc_in")
            vc_out = dp.tile([4, 128, VB], f8, name="vc_out")
            wk_all = pB.tile([128, NCH, HP, 128], bf16, tag="wk_all", bufs=1)
            nc.sync.dma_start(out=wk_all[:], in_=dram["wk"][:])
            wv_all = pB.tile([128, NCH, NCH, 128], bf16, tag="wv_all", bufs=1)
            nc.sync.dma_start(out=wv_all[:], in_=dram["wv"][:])
            Vf8 = pBw.tile([128, NKB, H, VW], f8, tag="w6")

            # ---- K projection, kick K AllGather ASAP (fp8, 96-row wire) ----
            Ktl = pB.tile([128, HP, QPC], f8, tag="Ktl", bufs=1)
            for hp in range(HP):
                ps = pBp.tile([128, QPC], f32, tag="ps")
                for ci in range(NCH):
                    nc.tensor.matmul(ps[:], wk_all[:, ci, hp, :],
                                     snT[:, ci, :],
                                     start=(ci == 0), stop=(ci == NCH - 1))
                nc.vector.tensor_copy(Ktl[:, hp, :], ps[:])
            nc.sync.dma_start(
                out=kc_in[0:48, :],
                in_=Ktl[0:48].rearrange("p a b -> p (a b)"))
            nc.sync.dma_start(
                out=kc_in[48:96, :],
                in_=Ktl[64:112].rearrange("p a b -> p (a b)"))
            nc.gpsimd.collective_compute(
                "AllGather", mybir.AluOpType.bypass,
                replica_groups=[[0, 1, 2, 3], [4, 5, 6, 7]],
                ins=[kc_in[:]], outs=[kc_out[:]])
            # unpack gathered K per head
            # (even heads ride wire rows 0..47, odd heads rows 48..95)
            for r in range(4):
                nc.sync.dma_start(
                    out=Kt128[0:48, 0 : H : 2, ts(r * QPC, QPC)],
                    in_=kc_out[r][0:48].rearrange("p (a b) -> p a b", a=HP))
                nc.sync.dma_start(
                    out=Kt128[0:48, 1 : H : 2, ts(r * QPC, QPC)],
                    in_=kc_out[r][48:96].rearrange("p (a b) -> p a b", a=HP))

            # ---- V projection into the ones-augmented layout, V AllGather --
            Vl49 = pB.tile([128, QT, H, VW], f8, tag="Vl49", bufs=1)
            nc.vector.memset(Vl49[:, :, :, DH : DH + 1], 1.0)
            for tt in range(QT):
                for cg in range(2):
                    psv = pBp.tile([128, 384], f32, tag="psv")
                    for ci in range(NCH):
                        nc.tensor.matmul(psv[:], snT[:, ci, ts(tt * 128, 128)],
                                         wv_all[:, ci, ts(cg * 3, 3)],
                                         start=(ci == 0), stop=(ci == NCH - 1))
                    nc.vector.tensor_copy(
                        Vl49[:, tt, ts(cg * 8, 8), 0:DH],
                        psv[:].rearrange("p (h d) -> p h d", h=8))
            nc.sync.dma_start(out=vc_in[:],
                              in_=Vl49[:].rearrange("p a h w -> p (a h w)"))
            nc.gpsimd.collective_compute(
                "AllGather", mybir.AluOpType.bypass,
                replica_groups=[[0, 1, 2, 3], [4, 5, 6, 7]],
                ins=[vc_in[:]], outs=[vc_out[:]])
            for r in range(4):
                nc.sync.dma_start(
                    out=Vf8[:, ts(r * QT, QT), :, :],
                    in_=vc_out[r].rearrange("p (a h w) -> p a h w",
                                            a=QT, h=H))
                nc.vector.tensor_copy(V49g[:, ts(r * QT, QT), :, :],
                                      Vf8[:, ts(r * QT, QT), :, :])

            # ---- Q projection (pair-packed psum -> per-head 128-row tiles) --
            Qt = pB.tile([128, HP, QPC], f8, tag="Qt", bufs=1)
            for hp in range(HP):
                wc = pB.tile([128, NCH, 128], bf16, tag="wc")
                nc.sync.dma_start(out=wc[:], in_=dram["wq"][:, hp, :, :])
                ps = pBp.tile([128, QPC], f32, tag="ps")
                for ci in range(NCH):
                    nc.tensor.matmul(ps[:], wc[:, ci, :], snT[:, ci, :],
                                     start=(ci == 0), stop=(ci == NCH - 1))
                nc.vector.tensor_copy(Qt[:, hp, :], ps[:])
            nc.sync.dma_start(
                out=Qt128[0:48, 0 : H : 2, :],
                in_=Qt[0:48, :, :])
            nc.sync.dma_start(
                out=Qt128[0:48, 1 : H : 2, :],
                in_=Qt[64:112, :, :])

            # ---- G gate ----
            wg_all = pBw.tile([128, NCH, NCH, 128], bf16, tag="w6")
            nc.sync.dma_start(out=wg_all[:], in_=dram["wg"][:])
            for co in range(NCH):
                psgf = pBp.tile([128, QPC], f32, tag="psgf")
                for ci in range(NCH):
                    nc.tensor.matmul(psgf[:], wg_all[:, ci, co, :],
                                     snT[:, ci, :],
                                     start=(ci == 0), stop=(ci == NCH - 1))
                nc.scalar.activation(sig_g[:, co, :], psgf[:], AF.Sigmoid)

            # ---- precompute g1 / g2 gates (cond-only) ----
            g1_all = pBw.tile([128, NCH, NCH, 128], bf16, tag="w6")
            nc.sync.dma_start(out=g1_all[:], in_=dram["g1w"][:])
            for co in range(NCH):
                ps1 = pBp.tile([128, QPC], f32, tag="ps")
                for ci in range(NCH):
                    nc.tensor.matmul(ps1[:], g1_all[:, ci, co, :],
                                     cT[:, ci, :],
                                     start=(ci == 0), stop=(ci == NCH - 1))
                nc.scalar.activation(sig1[:, co, :], ps1[:], AF.Sigmoid,
                                     bias=cvec["g1b"][:, co, :])
            g2_all = pBw.tile([128, NCH, NCH, 128], bf16, tag="w6")
            nc.sync.dma_start(out=g2_all[:], in_=dram["g2w"][:])
            for co in range(NCH):
                ps2 = pBp.tile([128, QPC], f32, tag="ps")
                for ci in range(NCH):
                    nc.tensor.matmul(ps2[:], g2_all[:, ci, co, :],
                                     cT[:, ci, :],
                                     start=(ci == 0), stop=(ci == NCH - 1))
                nc.scalar.activation(sig2[:, co, :], ps2[:], AF.Sigmoid,
                                     bias=cvec["g2b"][:, co, :])

            # ---- precompute AdaLN2 cond-side matmuls ----
            a2gw_all = pBw.tile([128, NCH, NCH, 128], bf16, tag="w6")
            nc.sync.dma_start(out=a2gw_all[:], in_=dram["a2gw"][:])
            for co in range(NCH):
                psg = pBp.tile([128, QPC], f32, tag="ps")
                for ci in range(NCH):
                    nc.tensor.matmul(psg[:], a2gw_all[:, ci, co, :],
                                     cn[:, ci, :],
                                     start=(ci == 0), stop=(ci == NCH - 1))
                nc.scalar.copy(psG2sb[:, co, :], psg[:])
            a2bw_all = pBw.tile([128, NCH, NCH, 128], bf16, tag="w6")
            nc.sync.dma_start(out=a2bw_all[:], in_=dram["a2bw"][:])
            for co in range(NCH):
                psb = pBp.tile([128, QPC], f32, tag="ps")
                for ci in range(NCH):
                    nc.tensor.matmul(psb[:], a2bw_all[:, ci, co, :],
                                     cn[:, ci, :],
                                     start=(ci == 0), stop=(ci == NCH - 1))
                nc.scalar.copy(psB2sb[:, co, :], psb[:])
            # premultiply the two phase-D gates off the critical path
            for co in range(NCH):
                nc.vector.tensor_mul(gate12[:, co, :], sig1[:, co, :],
                                     sig_g[:, co, :])

        snstack.close()  # free snT

        # ==================================================================
        # Phase C: attention (S^T layout) -> att_nT
        # ==================================================================
        dstack = contextlib.ExitStack()
        pDw = dstack.enter_context(tc.tile_pool(name="pDw", bufs=1))
        att_nT = pAtt.tile([128, HP, QPC], bf16, tag="att_nT")
        attU = pAtt.tile([128, HP, QPC], bf16, tag="attU")
        nc.gpsimd.memset(attU[:], 0.0)
        # prefetch phase-D operands ahead of the E-table DMA stream
        wo_all = pDw.tile([128, HP, NCH, 128], bf16, tag="wo_all")
        nc.sync.dma_start(out=wo_all[:], in_=dram["wo"][:])
        sqT = pDw.tile([128, NCH, QPC], f32, tag="sqT")
        nc.sync.dma_start(out=sqT[:], in_=dram["sqT"][:])
        Dstage = pAtt.tile([128, HP, QPC], bf16, tag="Dstage")
        Dall = pAtt.tile([16, QPC], bf16, tag="Dall")
        Dinv = pAtt.tile([16, QPC], bf16, tag="Dinv")

        # per-head key-block groups of 4/2 blocks, alternating through two
        # PSUM pools (4 + 2 banks) so consecutive groups' matmuls stay
        # decoupled from the exp drain (deep PE/ACT pipelining).
        GRP = [(0, 4), (4, 2), (6, 4), (10, 2)]
        with tc.tile_pool(name="pEt", bufs=7) as pEt, \
             tc.tile_pool(name="pPt", bufs=3) as pPt, \
             tc.tile_pool(name="pP2", bufs=5) as pP2, \
             tc.tile_pool(name="psS4", bufs=1, space="PSUM") as psS4, \
             tc.tile_pool(name="psS2", bufs=1, space="PSUM") as psS2, \
             tc.tile_pool(name="psPV", bufs=2, space="PSUM") as psPVp:
            for hp in range(HP):
                psPV = psPVp.tile([128, QPC], f32, tag="pv", name="pv")
                for s in range(2):
                    h, plo = 2 * hp + s, DHP * s
                    for gi, (kb0, nkb) in enumerate(GRP):
                        Et = pEt.tile([128, 4 * QPC], bf16, tag="Et")
                        Etv = Et[:, 0 : nkb * QPC]
                        nc.sync.dma_start(
                            out=Etv.rearrange("p (a b) -> p a b", b=QPC),
                            in_=dram["E"][h][:, ts(kb0, nkb), :])
                        pool = psS4 if nkb == 4 else psS2
                        psS = pool.tile([128, nkb * 512], f32,
                                        tag="sg", name="sg")
                        for j in range(nkb):
                            kb = kb0 + j
                            nc.tensor.matmul(
                                psS[:, ts(j * 512, QPC)],
                                Kt128[:, h, ts(kb * 128, 128)],
                                Qt128[:, h, :],
                                start=True, stop=True)
                        Pt = pPt.tile([128, 4 * QPC], bf16, tag="Pt")
                        nc.scalar.activation(
                            Pt[:, 0 : nkb * QPC].rearrange(
                                "p (a b) -> p a b", b=QPC),
                            psS[:].rearrange("p (a b) -> p a b", b=512)
                               [:, :, 0:QPC],
                            AF.Exp)
                        P2 = pP2.tile([128, 4 * QPC], bf16, tag="P2")
                        # small groups' multiplies go to the idle GpSimd
                        mul_eng = nc.vector if nkb == 4 else nc.gpsimd
                        mul_eng.tensor_mul(P2[:, 0 : nkb * QPC],
                                           Pt[:, 0 : nkb * QPC], Etv)
                        P2v = P2[:, 0 : nkb * QPC].rearrange(
                            "p (a b) -> p a b", b=QPC)
                        for j in range(nkb):
                            kb = kb0 + j
                            nc.tensor.matmul(
                                psPV[plo : plo + VW, :],
                                V49g[:, kb, h, :],
                                P2v[:, j, :],
                                start=(gi == 0 and j == 0),
                                stop=(gi == len(GRP) - 1 and j == nkb - 1),
                                tile_position=(0, plo),
                                skip_group_check=True)
                # drain denominators + unnormalized attention
                # (DVE base partition must be 32-aligned; rows 32..48 and
                #  96..112 are all PV-written, den sits at 48 / 112)
                nc.vector.tensor_copy(Dstage[32:49, hp, :], psPV[32:49, :])
                nc.vector.tensor_copy(Dstage[96:113, hp, :],
                                      psPV[96:113, :])
                for s in range(2):
                    plo = DHP * s
                    nc.vector.tensor_copy(attU[plo : plo + DH, hp, :],
                                          psPV[plo : plo + DH, :])
            # tail: batched reciprocal + per-pair broadcast + normalize
            nc.sync.dma_start(out=Dall[0:8, :], in_=Dstage[48:49, :, :])
            nc.sync.dma_start(out=Dall[8:16, :], in_=Dstage[112:113, :, :])
            with nc.allow_low_precision(reason="bf16 softmax denominators"):
                nc.vector.reciprocal(Dinv[:], Dall[:])
            for hp in range(HP):
                psb = psPVp.tile([128, QPC], f32, tag="pv", name="db")
                nc.tensor.matmul(psb[:], selm_sb[:, hp, :], Dinv[:],
                                 start=True, stop=True)
                nc.vector.tensor_mul(att_nT[:, hp, :], attU[:, hp, :],
                                     psb[:])

        # ==================================================================
        # Phase D: wo + gates + residual -> s_new
        # ==================================================================
        sn2 = outer.tile([128, NCH, QPC], f8, tag="sn2")
        xb2 = outer.tile([128, NCH, QPC], bf16, tag="xb2")
        sq2 = outer.tile([128, NCH, QPC], bf16, tag="sq2")
        with tc.tile_pool(name="pD", bufs=2) as pD, \
             tc.tile_pool(name="pDp", bufs=2, space="PSUM") as pDp:
            for co in range(NCH):
                pso = pDp.tile([128, QPC], f32, tag="pso")
                for ci in range(HP):
                    nc.tensor.matmul(pso[:], wo_all[:, ci, co, :],
                                     att_nT[:, ci, :],
                                     start=(ci == 0), stop=(ci == HP - 1))
                t2 = pD.tile([128, QPC], bf16, tag="t2")
                nc.vector.tensor_mul(t2[:], gate12[:, co, :], pso[:])
                nc.vector.tensor_add(s_new[:, co, :], sqT[:, co, :], t2[:])
                # feed the AdaLN2 stats incrementally
                nc.vector.tensor_copy(xb2[:, co, :], s_new[:, co, :])
                nc.vector.tensor_mul(sq2[:, co, :], xb2[:, co, :],
                                     xb2[:, co, :])

        dstack.close()   # free wo_all/sqT
        attstack.close()  # free snT/Kt/Qt/V49g/sig_g/sig1/att tiles

        # ==================================================================
        # Phase E: AdaLN2 (combine with precomputed cond matmuls) -> sn2
        # ==================================================================
        with tc.tile_pool(name="pE", bufs=1) as pE, \
             tc.tile_pool(name="pEt2", bufs=3) as pEt2, \
             tc.tile_pool(name="pEp", bufs=2, space="PSUM") as pEp:
            xn2 = pE.tile([128, NCH, QPC], bf16, tag="xn2")
            Rs2 = pE.tile([128, QPC], f32, tag="Rs2")
            Mb2 = pE.tile([128, QPC], f32, tag="Mb2")
            ln_stats(xb2, Mb2, Rs2, "s2", sq_pre=sq2)
            with tc.tile_pool(name="bcEw", bufs=3) as bw2:
                ln_apply(s_new, Mb2, Rs2, xn2, bw2)
            gb, bb = cvec["a2gb"], cvec["a2bb"]
            for co in range(NCH):
                sig = pEt2.tile([128, QPC], bf16, tag="sig")
                nc.scalar.activation(sig[:], psG2sb[:, co, :], AF.Sigmoid,
                                     bias=gb[:, co, :])
                t1 = pEt2.tile([128, QPC], bf16, tag="t1")
                nc.vector.tensor_mul(t1[:], sig[:], xn2[:, co, :])
                nc.vector.scalar_tensor_tensor(
                    sn2[:, co, :], psB2sb[:, co, :], bb[:, co, :],
                    t1[:], op0=OP.add, op1=OP.add)

        # ==================================================================
        # Phase F: SwiGLU + g2 gate + residual -> outT
        # ==================================================================
        with tc.tile_pool(name="pF", bufs=3) as pF, \
             tc.tile_pool(name="pFh", bufs=1) as pFh, \
             tc.tile_pool(name="pFp", bufs=2, space="PSUM") as pFp:
            DR = mybir.MatmulPerfMode.DoubleRow
            hT = pFh.tile([128, FCH, QPC], f8, tag="hT")
            for co in range(FCH):
                gwc = pF.tile([128, NCH, 128], f8, tag="gwc")
                nc.sync.dma_start(out=gwc[:], in_=dram["swg"][:, co, :, :])
                uwc = pF.tile([128, NCH, 128], f8, tag="uwc")
                nc.sync.dma_start(out=uwc[:], in_=dram["swu"][:, co, :, :])
                psG = pFp.tile([128, QPC], f32, tag="psG")
                psU = pFp.tile([128, QPC], f32, tag="psU")
                for c in range(NCH // 2):
                    nc.tensor.matmul(psG[:], gwc[:, ts(2 * c, 2), :],
                                     sn2[:, ts(2 * c, 2), :],
                                     start=(c == 0), stop=(c == NCH // 2 - 1),
                                     perf_mode=DR)
                    nc.tensor.matmul(psU[:], uwc[:, ts(2 * c, 2), :],
                                     sn2[:, ts(2 * c, 2), :],
                                     start=(c == 0), stop=(c == NCH // 2 - 1),
                                     perf_mode=DR)
                sg = pF.tile([128, QPC], bf16, tag="sg")
                nc.scalar.activation(sg[:], psG[:], AF.Sigmoid, scale=1.0 / 128)
                tg = pF.tile([128, QPC], bf16, tag="tg")
                nc.vector.scalar_tensor_tensor(
                    tg[:], psG[:], 1.0 / 128, sg[:],
                    op0=OP.mult, op1=OP.mult)
                nc.vector.scalar_tensor_tensor(
                    hT[:, co, :], psU[:], 1.0 / 128, tg[:],
                    op0=OP.mult, op1=OP.mult)
            outT = pFh.tile([128, NCH, QPC], f32, tag="outT")
            for co in range(NCH):
                dwc = pF.tile([128, FCH, 128], f8, tag="dwc")
                nc.sync.dma_start(out=dwc[:], in_=dram["swd"][:, co, :, :])
                psD = pFp.tile([128, QPC], f32, tag="psD")
                for c in range(FCH // 2):
                    nc.tensor.matmul(psD[:], dwc[:, ts(2 * c, 2), :],
                                     hT[:, ts(2 * c, 2), :],
                                     start=(c == 0), stop=(c == FCH // 2 - 1),
                                     perf_mode=DR)
                t3 = pF.tile([128, QPC], bf16, tag="t3")
                nc.vector.scalar_tensor_tensor(
                    t3[:], psD[:], 1.0 / 128, sig2[:, co, :],
                    op0=OP.mult, op1=OP.mult)
                nc.vector.tensor_add(outT[:, co, :], s_new[:, co, :], t3[:])
            nc.sync.dma_start(out=dram["outT"][:], in_=outT[:])


# ----------------------------------------------------------------------------
# public entry point
# ----------------------------------------------------------------------------

def get_program():
    if "nc" not in _PROGRAM_CACHE:
        _PROGRAM_CACHE["nc"] = build_program()
    return _PROGRAM_CACHE["nc"]


def kernel(**inputs):
    from concourse.bass_utils import run_bass_kernel_spmd

    nc = get_program()
    in_maps = host_prep(inputs)
    res = run_bass_kernel_spmd(nc, in_maps, list(range(NCORES)))
    return assemble_output(res.results)


if __name__ == "__main__":
    import reference

    inputs = {k: np.asarray(v) for k, v in reference.setup_inputs().items()}
    out = kernel(**inputs)
    print("kernel output", out.shape, out.dtype)
